# revision 21
# baseline (speedup 1.0000x reference)
"""Trainium2 Bass kernel for DifferentiableToposAttention.

Math:
  Q = sigmoid(x @ Wq.T + bq); K = sigmoid(x @ Wk.T + bk); V = x @ Wv.T + bv
  truth[q,k] = mean_d min(1 - Q[q,d] + K[k,d], 1) = 1 - (1/D) sum_d relu(Q-K)
  logit[q,k] = 10*truth; masked (k>q) logits are 0 exactly (weight exp(0)=1).
  out[q,:] = sum_k softmax(logit)[q,k] * V[k,:]

Score via PWL-interpolated relu as a matmul (contraction D*T, T=8):
  relu(a-b) ~= sum_{m=1..T} G_m(a) * r_m(b)
  r_m(b) = relu(m/T - b)                     (ACT, one op per m)
  G_m(a) = L_{m-1}(a) - L_m(a),  G_T = L_{T-1},
  L_m(a) = clamp(T*a - m, 0, 1)              (DVE, two ops per m)
This is exact PWL interpolation of relu(a-b) on the a-grid {m/T}; the
only error is Q-side quantization (<= 1/(2T), active only for same-cell
pairs).  End-to-end rel err vs fp32 reference ~4e-3 at T=8.

Sharding: 8 cores; core c = (b=c//4, l=c%4) handles batch b, query
blocks gA=l (keys window [0,512)) and gB=7-l (window [0,1024)) -- the
same compile-time shapes on every core (SPMD), host routes data.
Scores in [k,q] orientation (PSUM [128k, q]) so the exp output E^T is
directly the AV stationary -- no transposes anywhere.  exp fuses the
logit scale/bias (ACT: exp(-10/D * s + 10)).  Causal mask applied
post-exp: E' = E*M1 + (1-M1), M1 host-built per core.  Keys beyond the
512 window of group A contribute an analytic suffix: one matmul adds
ones[q] (x) [suffix-V | count] into the AV PSUM, whose appended ones
column accumulates the softmax denominator alongside AV.
"""

import sys

for _p in ("/opt/trn_rl_repo",):
    if _p not in sys.path:
        sys.path.insert(0, _p)

import numpy as np

import concourse.bass as bass
import concourse.mybir as mybir
import concourse.tile as tile
from concourse import bacc
from concourse.bass import ts
from concourse.bass_utils import run_bass_kernel_spmd

F32 = mybir.dt.float32
FP16 = mybir.dt.float16
AF = mybir.ActivationFunctionType
ALU = mybir.AluOpType

B, S, D = 2, 1024, 128
NCORES = 8
T = 8          # PWL knots
NKB = 8        # key blocks of 128


def _kb_map(kb: int, masked: bool):
    """kb -> (bank, col offset in bank, q-width). Masked: kb<4 carries
    both query groups (w=256), kb>=4 only group B (w=128)."""
    if masked:
        if kb < 4:
            return kb // 2, (kb % 2) * 256, 256
        return 2, (kb - 4) * 128, 128
    return kb // 2, (kb % 2) * 256, 256


def _build_program(masked: bool) -> bass.Bass:
    nbanks = 3 if masked else 4
    nc = bacc.Bacc()

    xT_d = nc.declare_dram_parameter("xT", [D, S], FP16, isOutput=False)
    xqT_d = nc.declare_dram_parameter("xqT", [D, 256], FP16, isOutput=False)
    wqt_d = nc.declare_dram_parameter("wqt", [D, D], FP16, isOutput=False)
    wkt_d = nc.declare_dram_parameter("wkt", [D, D], FP16, isOutput=False)
    wvt_d = nc.declare_dram_parameter("wvt", [D, D], FP16, isOutput=False)
    bq_d = nc.declare_dram_parameter("bq", [D, 1], F32, isOutput=False)
    bk_d = nc.declare_dram_parameter("bk", [D, 1], F32, isOutput=False)
    bvb_d = nc.declare_dram_parameter("bvb", [D, 4 * D], F32, isOutput=False)
    cb_d = nc.declare_dram_parameter("cb", [D, T + 1], F32, isOutput=False)
    if masked:
        m1_d = nc.declare_dram_parameter(
            "m1", [128, nbanks * 512], FP16, isOutput=False)
    out_d = nc.declare_dram_parameter("out", [256, D], F32, isOutput=True)

    with tile.TileContext(nc) as tc:
        with tc.tile_pool(name="singles", bufs=1) as singles:
            ones_col = singles.tile([128, 1], FP16)
            nc.vector.memset(ones_col[:], 1.0)
            ones_row = singles.tile([1, 128], FP16)
            nc.vector.memset(ones_row[:], 1.0)
            cb_sb = singles.tile([128, T + 1], F32)  # [m/T biases | 10.0]

            wq_sb = singles.tile([128, 128], FP16)
            wk_sb = singles.tile([128, 128], FP16)
            wv_sb = singles.tile([128, 128], FP16)
            bq_sb = singles.tile([128, 1], F32)
            bk_sb = singles.tile([128, 1], F32)
            bvb4_sb = singles.tile([128, 4 * 128], F32)
            xT = singles.tile([128, S], FP16)
            xqT = singles.tile([128, 256], FP16)
            KTb = singles.tile([128, S], FP16)
            QTb = singles.tile([128, 256], FP16)
            r_all = singles.tile([128, T, S], FP16)      # r_m(K)
            L_all = singles.tile([128, T, 256], FP16)    # clamp ramps of Q
            G_all = singles.tile([128, T, 256], FP16)    # tents of Q
            Vhat = singles.tile([128, NKB, D + 1], FP16)  # [V | 1]
            E_raw = singles.tile([128, nbanks, 512], FP16)
            outA_s = singles.tile([128, 128], F32)
            outB_s = singles.tile([128, 128], F32)
            rcpA = singles.tile([128, 1], F32)
            rcpB = singles.tile([128, 1], F32)
            if masked:
                E2 = singles.tile([128, nbanks, 512], FP16)
                m1_sb = singles.tile([128, nbanks * 512], FP16)
                m2_sb = singles.tile([128, nbanks * 512], FP16)
                sfx_row = singles.tile([1, D + 1], FP16)

            nc.sync.dma_start(out=wk_sb[:], in_=wkt_d[:, :])
            nc.sync.dma_start(out=xT[:, 0:512], in_=xT_d[:, 0:512])
            nc.sync.dma_start(out=xT[:, 512:1024], in_=xT_d[:, 512:1024])
            nc.sync.dma_start(out=cb_sb[:], in_=cb_d[:, :])
            nc.sync.dma_start(out=wq_sb[:], in_=wqt_d[:, :])
            nc.sync.dma_start(out=xqT[:], in_=xqT_d[:, :])
            nc.sync.dma_start(out=bk_sb[:], in_=bk_d[:, :])
            nc.sync.dma_start(out=bq_sb[:], in_=bq_d[:, :])
            nc.gpsimd.dma_start(out=wv_sb[:], in_=wvt_d[:, :])
            nc.gpsimd.dma_start(out=bvb4_sb[:], in_=bvb_d[:, :])
            if masked:
                nc.gpsimd.dma_start(out=m1_sb[:], in_=m1_d[:, :])
            nc.vector.memset(Vhat[:, :, D:D + 1], 1.0)

            # ---- phase A: projections + encodings ----
            with (
                tc.tile_pool(name="pp", bufs=2, space="PSUM") as pp,
                tc.tile_pool(name="pv", bufs=2, space="PSUM") as pv,
                tc.tile_pool(name="psf", bufs=1, space="PSUM") as psf,
            ):
                # K^T = sigmoid(Wk @ x^T + bk)
                psK = pp.tile([128, 512], F32, tag="p")
                nc.tensor.matmul(psK[:], wk_sb[:], xT[:, 0:512])
                nc.scalar.activation(
                    KTb[:, 0:512], psK[:], AF.Sigmoid, bias=bk_sb[:], scale=1.0)
                psK2 = pp.tile([128, 512], F32, tag="p")
                nc.tensor.matmul(psK2[:], wk_sb[:], xT[:, 512:1024])
                nc.scalar.activation(
                    KTb[:, 512:1024], psK2[:], AF.Sigmoid, bias=bk_sb[:],
                    scale=1.0)
                # Q^T for the core's 256 queries
                psQ = pp.tile([128, 512], F32, tag="p")
                nc.tensor.matmul(psQ[:, 0:256], wq_sb[:], xqT[:])
                nc.scalar.activation(
                    QTb[:], psQ[:, 0:256], AF.Sigmoid, bias=bq_sb[:], scale=1.0)

                # warm the Exp table now so it doesn't stall the tail
                exp_warm = pp.tile([128, 1], F32, tag="w")
                nc.scalar.activation(exp_warm[:], bq_sb[:], AF.Exp)

                # r_m(K) = relu(m/T - K): odd m on ACT (1 op), even m on
                # DVE (2 ops) so neither engine pegs.
                def emit_r(m):
                    if m % 2 == 1:
                        nc.scalar.activation(
                            r_all[:, m - 1, :], KTb[:], AF.Relu,
                            bias=cb_sb[:, m - 1:m], scale=-1.0)
                    else:
                        nc.vector.tensor_scalar(
                            r_all[:, m - 1, :], KTb[:], -1.0, float(m) / T,
                            ALU.mult, ALU.add)
                        nc.vector.tensor_scalar(
                            r_all[:, m - 1, :], r_all[:, m - 1, :], 0.0, None,
                            ALU.max)

                emit_r(1)
                emit_r(2)

                # L_m(Q) = clamp(T*Q - m, 0, 1); G_m = L_{m-1} - L_m
                for m in range(T):
                    nc.vector.tensor_scalar(
                        L_all[:, m, :], QTb[:], float(T), float(-m),
                        ALU.mult, ALU.add)
                    nc.vector.tensor_scalar(
                        L_all[:, m, :], L_all[:, m, :], 0.0, 1.0,
                        ALU.max, ALU.min)
                    if m >= 1:
                        nc.vector.tensor_sub(
                            G_all[:, m - 1, :], L_all[:, m - 1, :],
                            L_all[:, m, :])
                nc.vector.tensor_copy(G_all[:, T - 1, :], L_all[:, T - 1, :])

                for m in range(3, T + 1):
                    emit_r(m)

                # V blocks (natural [k, e]) + bias on gpsimd (idle engine)
                for half in range(2):
                    psV = pv.tile([128, 4, 128], F32, tag="v")
                    for i in range(4):
                        kb = half * 4 + i
                        nc.tensor.matmul(
                            psV[:, i, :], xT[:, ts(kb, 128)], wv_sb[:])
                    nc.vector.tensor_add(
                        Vhat[:, ts(half, 4), 0:D],
                        psV[:], bvb4_sb[:])

                if masked:
                    # suffix over key blocks 4..7: [sum V | count=512]
                    pssfx = psf.tile([1, D + 1], F32, tag="s")
                    for kb in range(4, 8):
                        nc.tensor.matmul(
                            pssfx[:], ones_col[:], Vhat[:, kb, :],
                            start=(kb == 4), stop=(kb == 7))
                    nc.scalar.copy(sfx_row[:], pssfx[:])
                    # M2 = 1 - M1 (gpsimd: off the DVE critical path)
                    nc.gpsimd.tensor_scalar(
                        m2_sb[:], m1_sb[:], -1.0, 1.0, ALU.mult, ALU.add)

            # ---- phase B: score matmuls + exp/mask + AV ----
            with (
                tc.tile_pool(name="psc", bufs=1, space="PSUM") as pscp,
                tc.tile_pool(name="pav", bufs=1, space="PSUM") as pav,
            ):
                psc = []
                for bk_ in range(nbanks):
                    sc_bank = pscp.tile([128, 512], F32, tag=f"sc{bk_}")
                    psc.append(sc_bank)

                def score_mm(m, kb):
                    bank, off, w = _kb_map(kb, masked)
                    nc.tensor.matmul(
                        psc[bank][:, off:off + w],
                        r_all[:, m - 1, ts(kb, 128)],
                        G_all[:, m - 1, 256 - w:256],
                        start=(m == 1), stop=(m == T),
                        skip_group_check=True)

                def finish_bank(bank):
                    # exp with fused logit scale/bias, then causal mask
                    nc.scalar.activation(
                        E_raw[:, bank, :], psc[bank][:], AF.Exp,
                        bias=cb_sb[:, T:T + 1], scale=-10.0 / D)
                    if masked:
                        nc.vector.tensor_mul(
                            E2[:, bank, :], E_raw[:, bank, :],
                            m1_sb[:, ts(bank, 512)])
                        nc.vector.tensor_add(
                            E2[:, bank, :], E2[:, bank, :],
                            m2_sb[:, ts(bank, 512)])

                for m in range(1, T):
                    for kb in range(NKB):
                        score_mm(m, kb)
                # final contraction pass bank-by-bank so exp/mask of a
                # finished bank overlaps the remaining matmuls
                kb_of_bank = [[kb for kb in range(NKB)
                               if _kb_map(kb, masked)[0] == bk_]
                              for bk_ in range(nbanks)]
                for bk_ in range(nbanks):
                    for kb in kb_of_bank[bk_]:
                        score_mm(T, kb)
                    finish_bank(bk_)
                E_use = E2 if masked else E_raw

                # AV + den: stationary = E' slice, moving = [V | 1]
                avA = pav.tile([128, D + 1], F32, tag="avA")
                avB = pav.tile([128, D + 1], F32, tag="avB")
                nblk_a = 4 if masked else 8
                for i, kb in enumerate(range(nblk_a)):
                    bank, off, w = _kb_map(kb, masked)
                    st = E_use[:, bank, off:off + 128]
                    nc.tensor.matmul(
                        avA[:], st, Vhat[:, kb, :],
                        start=(i == 0), stop=(not masked and kb == nblk_a - 1),
                        skip_group_check=True)
                if masked:
                    nc.tensor.matmul(
                        avA[:], ones_row[:], sfx_row[:],
                        start=False, stop=True, skip_group_check=True)
                for kb in range(NKB):
                    bank, off, w = _kb_map(kb, masked)
                    st = E_use[:, bank, off + w - 128:off + w]
                    nc.tensor.matmul(
                        avB[:], st, Vhat[:, kb, :],
                        start=(kb == 0), stop=(kb == NKB - 1),
                        skip_group_check=True)

                nc.vector.reciprocal(rcpA[:], avA[:, D:D + 1])
                nc.vector.tensor_scalar(
                    outA_s[:], avA[:, 0:D], rcpA[:], None, ALU.mult)
                nc.sync.dma_start(out=out_d[0:128, :], in_=outA_s[:])
                nc.vector.reciprocal(rcpB[:], avB[:, D:D + 1])
                nc.vector.tensor_scalar(
                    outB_s[:], avB[:, 0:D], rcpB[:], None, ALU.mult)
                nc.sync.dma_start(out=out_d[128:256, :], in_=outB_s[:])

    nc.finalize()
    return nc


_PROG_CACHE: dict[bool, bass.Bass] = {}


def _get_program(masked: bool) -> bass.Bass:
    if masked not in _PROG_CACHE:
        _PROG_CACHE[masked] = _build_program(masked)
    return _PROG_CACHE[masked]


def _build_m1(l: int) -> np.ndarray:
    """Post-exp causal mask, [k,q] orientation, bank-packed [128, 1536].
    1 = keep computed weight, 0 = masked (weight forced to exp(0)=1)."""
    gA, gB = l, 7 - l
    m1 = np.zeros((128, 3 * 512), dtype=np.float16)
    tri = (np.arange(128)[:, None] <= np.arange(128)[None, :])  # k<=q in blk
    for kb in range(8):
        bank, off, w = _kb_map(kb, True)
        base = bank * 512 + off
        units = [(gA, base), (gB, base + 128)] if w == 256 else [(gB, base)]
        for g, col in units:
            if kb < g:
                m1[:, col:col + 128] = 1.0
            elif kb == g:
                m1[:, col:col + 128] = tri
    return m1


def build_in_maps(x, Wq, bq, Wk, bk, Wv, bv, masked):
    wqt = np.ascontiguousarray(Wq.T.astype(np.float16))
    wkt = np.ascontiguousarray(Wk.T.astype(np.float16))
    wvt = np.ascontiguousarray(Wv.T.astype(np.float16))
    bq2 = np.ascontiguousarray(bq.reshape(D, 1).astype(np.float32))
    bk2 = np.ascontiguousarray(bk.reshape(D, 1).astype(np.float32))
    bvb = np.ascontiguousarray(
        np.tile(bv.reshape(1, D).astype(np.float32), (D, 4)))
    cb = np.tile(np.concatenate(
        [(np.arange(1, T + 1, dtype=np.float32)) / T,
         np.array([10.0], np.float32)]).reshape(1, T + 1), (D, 1))
    cb = np.ascontiguousarray(cb)
    in_maps = []
    xTs = [np.ascontiguousarray(x[b].T.astype(np.float16)) for b in range(B)]
    for c in range(NCORES):
        b, l = divmod(c, 4)
        gA, gB = l, 7 - l
        xT = xTs[b]
        xqT = np.ascontiguousarray(
            np.concatenate(
                [xT[:, 128 * gA:128 * gA + 128],
                 xT[:, 128 * gB:128 * gB + 128]], axis=1))
        im = {"xT": xT, "xqT": xqT, "wqt": wqt, "wkt": wkt, "wvt": wvt,
              "bq": bq2, "bk": bk2, "bvb": bvb, "cb": cb}
        if masked:
            im["m1"] = _build_m1(l)
        in_maps.append(im)
    return in_maps


def assemble_out(results, masked):
    out = np.empty((B, S, D), dtype=np.float32)
    for c in range(NCORES):
        b, l = divmod(c, 4)
        gA, gB = l, 7 - l
        out[b, 128 * gA:128 * gA + 128] = results[c]["out"][0:128]
        out[b, 128 * gB:128 * gB + 128] = results[c]["out"][128:256]
    return out


def kernel(x, Wq, bq, Wk, bk, Wv, bv, apply_causal_mask):
    x = np.ascontiguousarray(np.asarray(x, dtype=np.float32))
    Wq = np.asarray(Wq, dtype=np.float32)
    Wk = np.asarray(Wk, dtype=np.float32)
    Wv = np.asarray(Wv, dtype=np.float32)
    bq = np.asarray(bq, dtype=np.float32)
    bk = np.asarray(bk, dtype=np.float32)
    bv = np.asarray(bv, dtype=np.float32)
    masked = bool(int(np.asarray(apply_causal_mask)))

    nc = _get_program(masked)
    in_maps = build_in_maps(x, Wq, bq, Wk, bk, Wv, bv, masked)
    res = run_bass_kernel_spmd(nc, in_maps, list(range(NCORES))).results
    return assemble_out(res, masked)
